# revision 31
# baseline (speedup 1.0000x reference)
"""GAT block (graph attention) Bass/Tile kernel for Trainium2, 8 NeuronCores.

Full-input contract: kernel(x=(8,2048,128), W=(128,64), a=(128,1)) -> (8,2048,64).
Sharding: data-parallel over batch — one batch element per core, W/a replicated.

Per-core math (N=2048, Fin=128, Fout=64):
  h  = x @ W                               (N, Fout)
  s1 = h @ a[:64, 0],  s2 = h @ a[64:, 0]  (N,)
  e[i, j]   = leakyrelu(s1[i] + s2[j], 0.2)
  att       = softmax(e, axis=0)  (normalize over i for each column j)
  out       = leakyrelu(att @ h, 0.2)

Implementation notes:
  * attention matrix kept transposed: Pt[j, i] = exp(lrelu(s1[i] + s2[j])).
    leakyrelu tiles are split across ACT and DVE(+GPSIMD) to balance engines;
    the exp runs on ACT (Prelu/parametric_relu shares the exp activation-table
    set so there are no table reloads) with accum_out giving the softmax
    denominator for free.
  * No max-subtraction: |s1+s2| <~ 15 so exp is far from fp32 overflow; this
    matches jax softmax to fp32 rounding.
  * setup matmuls run in float32r (single-pass) — fp32 matmuls on TRN2 are
    two-pass (LOW/HIGH) and twice the cost.  h and both score projections
    come from one stationary [W | W@a1 | W@a2] per x-tile.
  * out is accumulated transposed (hpT[f, i] in 4 PSUM banks, one per
    512-wide i-chunk) so the 64 bf16 matmuls overlap the ACT/DVE stream
    tile-by-tile; the host un-transposes the (64, 2048) result.
"""

import numpy as np
from contextlib import ExitStack

import concourse.bass as bass
import concourse.mybir as mybir
import concourse.tile as tile
from concourse.tile import add_dep_helper
from concourse import bacc
from concourse._compat import with_exitstack
from concourse.bass_utils import run_bass_kernel_spmd
from concourse.masks import make_identity

# ---- custom DVE op: out = max(in0 + s0, (in0 + s0) * imm2) — fused
# leakyrelu-with-per-partition-bias, one DVE instruction per tile ----
import numpy as _np
from concourse import dve_ops as _dvo
from concourse.dve_spec import Spec as _Spec, Src0 as _Src0, C0 as _C0, C2 as _C2
from concourse.dve_spec import maxx as _maxx, lower as _dve_lower
from concourse.dve_spec import _has_src1 as _dve_has_src1
from concourse.dve_uop import DveOpSpec as _DveOpSpec
from concourse.dve_table_gen import dve_ver_for as _dve_ver_for


def _register_lrelu_bias():
    name = "LRELU_BIAS_GAT_ANT"
    if name in _dvo._SUB_OPCODE_FOR_NAME:
        return next(o for o in _dvo.OPS if o.name == name)
    spec = _Spec(
        body=_maxx(_Src0 + _C0, (_Src0 + _C0) * _C2),
        reference=lambda in0, in1, s0, s1, imm2: _np.maximum(
            in0.astype(_np.float32) + s0, (in0.astype(_np.float32) + s0) * imm2
        ).astype(_np.float32),
    )
    op = _dvo.DveOp(name, spec, subdim=False, uops_sha={},
                    perf_en={"v3": True, "v4": True})
    row = _dvo._CUSTOM_DVE_ROW_BASE + len(_dvo.OPS)
    assert row < 0x20
    _dvo.OPS.append(op)
    _dvo.CUSTOM_DVE_SPECS[name] = spec
    _dvo._SUB_OPCODE_FOR_NAME[name] = row
    for ver in ("v3", "v4"):
        try:
            s = _DveOpSpec(name=name, opcode=row, uops=_dve_lower(spec, ver=ver),
                           rd1_en=_dve_has_src1(spec)).sha(ver)
            op.uops_sha[ver] = s
        except Exception:
            pass
    return op


_LRELU_BIAS = _register_lrelu_bias()

F32 = mybir.dt.float32
F32R = mybir.dt.float32r
BF16 = mybir.dt.bfloat16
AF = mybir.ActivationFunctionType
ALU = mybir.AluOpType

N = 2048
FIN = 128
FOUT = 64
P = 128
T = N // P          # 16 row tiles
NC = N // 512       # 4 i-chunks for the output accumulation
NEG_SLOPE = 0.2
N_CORES = 8


@with_exitstack
def _gat_body(ctx: ExitStack, tc: tile.TileContext, x, w, a, out):
    nc = tc.nc

    const = ctx.enter_context(tc.tile_pool(name="const", bufs=1))
    xin = ctx.enter_context(tc.tile_pool(name="xin", bufs=16))
    lpool = ctx.enter_context(tc.tile_pool(name="lrelu", bufs=6))
    dpool = ctx.enter_context(tc.tile_pool(name="denoms", bufs=2 * T))

    # ---- constants / persistent tiles ----
    ident = const.tile([P, P], F32)
    make_identity(nc, ident)
    w_raw = const.tile([FIN, FOUT], F32)
    nc.sync.dma_start(w_raw[:], w)
    a_raw = const.tile([FOUT, 2], F32)  # [:,0]=a1, [:,1]=a2
    nc.sync.dma_start(a_raw[:, 0:1], a[0:FOUT, :])
    nc.sync.dma_start(a_raw[:, 1:2], a[FOUT:, :])
    acol = const.tile([FOUT, 2], F32R)
    nc.vector.tensor_copy(acol[:], a_raw[:])
    ones_raw = const.tile([1, P], F32)
    nc.vector.memset(ones_raw[:], 1.0)
    ones_row = const.tile([1, P], F32R)
    nc.vector.tensor_copy(ones_row[:], ones_raw[:])

    xT = const.tile([P, T, P], F32R)        # x transposed: [k, t, n]
    hs12 = const.tile([P, T, FOUT + 2], F32)  # [h | s1 s2 cols] per tile
    hs_bf = const.tile([P, T, FOUT], BF16)  # h/denom in bf16
    wsa = const.tile([FIN, FOUT + 2], F32R)  # [W | W@a1 | W@a2]
    s1b = const.tile([P, N], F32)           # s1 broadcast along partitions
    srow = const.tile([2, N], F32R)         # [s1, s2] as rows
    p_all = const.tile([P, T, N], BF16)     # attention numerator, transposed
    o_sb = const.tile([FOUT, N], F32)       # output transposed

    with tc.tile_pool(name="ps_tr", bufs=2, space="PSUM") as ps_tr, \
         tc.tile_pool(name="ps_mm", bufs=2, space="PSUM") as ps_mm:
        # ~5us of junk bf16 matmuls so the PE HAM un-throttles (1.2->2.4GHz)
        # before the real setup matmuls; overlaps the x DMA wait.
        wup = const.tile([P, 512], BF16)
        nc.vector.memset(wup[:], 0.0)
        for i in range(7):
            ps_w = ps_mm.tile([P, 512], F32, tag="ps_h")
            nc.tensor.matmul(ps_w[:], lhsT=wup[:, 0:P], rhs=wup[:],
                             start=True, stop=True)

        # wsa = [W | W @ [a1, a2]]  (wa via wT = W.T, contraction over f)
        nc.vector.tensor_copy(wsa[:, 0:FOUT], w_raw[:])
        ps_wT = ps_mm.tile([FOUT, FIN], F32, tag="ps_h")
        nc.tensor.transpose(ps_wT[:], w_raw[:], ident[:])
        wT = const.tile([FOUT, FIN], F32R)
        nc.vector.tensor_copy(wT[:], ps_wT[:])
        ps_wa = ps_mm.tile([FIN, 2], F32, tag="ps_r")
        nc.tensor.matmul(ps_wa[:], lhsT=wT[:], rhs=acol[:], start=True, stop=True)
        nc.vector.tensor_copy(wsa[:, FOUT:], ps_wa[:])

        # Score path first (it gates the whole exp stream): per tile,
        # transpose; one tile behind, the tiny srow matmul + bcast chunks.
        # The h matmuls are emitted afterwards — they only matter mid-stream.
        score_done = {}

        def load_transpose(t):
            xn = xin.tile([P, FIN], F32, tag="xn", name=f"xn{t}")
            dma_eng = (nc.sync, nc.gpsimd)[t % 2]
            dma_eng.dma_start(xn[:], x[t * P:(t + 1) * P, :])
            psT = ps_tr.tile([P, P], F32, tag="ps_t", name=f"psT{t}")
            nc.tensor.transpose(psT[:], xn[:], ident[:])
            # f32->f32r rounding copy, split across ACT/DVE
            if t % 2 == 0:
                score_done["act"] = nc.scalar.copy(xT[:, t, :], psT[:])
            else:
                score_done["dve"] = nc.vector.tensor_copy(xT[:, t, :], psT[:])

        def srow_mm(t):
            ps_r = ps_mm.tile([2, P], F32, tag="ps_r", name=f"ps_r{t}")
            score_done["pe"] = nc.tensor.matmul(
                ps_r[:], lhsT=wsa[:, FOUT:], rhs=xT[:, t, :],
                start=True, stop=True)
            if t % 2 == 0:
                score_done["act"] = nc.scalar.copy(
                    srow[:, t * P:(t + 1) * P], ps_r[:])
            else:
                score_done["dve"] = nc.vector.tensor_copy(
                    srow[:, t * P:(t + 1) * P], ps_r[:])
            if t % 4 == 3:
                c = t // 4
                sl = slice(c * 512, (c + 1) * 512)
                ps_b = ps_mm.tile([P, 512], F32, tag="ps_b", name=f"ps_b{c}")
                score_done["pe"] = nc.tensor.matmul(
                    ps_b[:], lhsT=ones_row[:], rhs=srow[0:1, sl],
                    start=True, stop=True)
                score_done["dve"] = nc.vector.tensor_copy(s1b[:, sl], ps_b[:])

        load_transpose(0)
        for t in range(1, T):
            load_transpose(t)
            srow_mm(t - 1)
        srow_mm(T - 1)

        # h matmuls only matter mid-stream: keep them strictly after the
        # score path on each engine so they can't dilute its pipeline.
        for t in range(T):
            ps_h = ps_mm.tile([P, FOUT + 2], F32, tag="ps_h", name=f"ps_h{t}")
            mm = nc.tensor.matmul(ps_h[:], lhsT=xT[:, t, :], rhs=wsa[:],
                                  start=True, stop=True)
            add_dep_helper(mm.ins, score_done["pe"].ins, sync=False,
                           reason="h-matmuls after score path")
            if t % 2 == 0:
                cp = nc.scalar.copy(hs12[:, t, :], ps_h[:])
                add_dep_helper(cp.ins, score_done["act"].ins, sync=False,
                               reason="h-copies after xT casts")
            else:
                cp = nc.vector.tensor_copy(hs12[:, t, :], ps_h[:])
                add_dep_helper(cp.ins, score_done["dve"].ins, sync=False,
                               reason="h-copies after score path")

    # setup PSUM pools released; output accumulators take the banks
    ps_out = ctx.enter_context(tc.tile_pool(name="ps_out", bufs=1, space="PSUM"))
    hp_ps = [ps_out.tile([FOUT, 512], F32, tag=f"hp{c}", name=f"hp{c}")
             for c in range(NC)]

    # ---- main: per j-tile lrelu -> exp(+denom) -> scale h -> accumulate out ----
    GROUPS = [(0, 4), (4, 4), (8, 4), (12, 2), (14, 1), (15, 1)]
    for g0, gn in GROUPS:
        den_g = dpool.tile([P, gn], F32, tag="den", name=f"den{g0}")
        for t in range(g0, g0 + gn):
            s2c = hs12[:, t, FOUT + 1:FOUT + 2]
            l_t = lpool.tile([P, N], F32, tag="l")
            nc.vector._custom_dve(_LRELU_BIAS, out=l_t[:], in0=s1b[:],
                                  s0=s2c, imm2=NEG_SLOPE)
            nc.scalar.activation(p_all[:, t, :], l_t[:],
                                 AF.Exp, accum_out=den_g[:, t - g0:t - g0 + 1])

        rden_g = dpool.tile([P, gn], F32, tag="rden", name=f"rden{g0}")
        nc.vector.reciprocal(rden_g[:], den_g[:])
        for u in range(g0, g0 + gn):
            rd = rden_g[:, u - g0:u - g0 + 1]
            if u % 2 == 0:
                nc.scalar.activation(hs_bf[:, u, :], hs12[:, u, 0:FOUT],
                                     AF.Copy, scale=rd)
            else:
                nc.vector.tensor_scalar_mul(hs_bf[:, u, :],
                                            hs12[:, u, 0:FOUT], rd)
            for c in range(NC):
                nc.tensor.matmul(hp_ps[c][:], lhsT=hs_bf[:, u, :],
                                 rhs=p_all[:, u, c * 512:(c + 1) * 512],
                                 start=(u == 0), stop=(u == T - 1))

    # ---- epilogue: leakyrelu on ACT straight from PSUM, DMA out transposed ----
    for c in range(NC):
        sl = slice(c * 512, (c + 1) * 512)
        nc.scalar.activation(o_sb[:, sl], hp_ps[c][:], AF.Prelu,
                             bias=0.0, scale=1.0, alpha=NEG_SLOPE)
        (nc.sync if c % 2 == 0 else nc.gpsimd).dma_start(out[:, sl], o_sb[:, sl])


_NC_CACHE = {}


def _build_nc():
    if "nc" in _NC_CACHE:
        return _NC_CACHE["nc"]
    nc = bacc.Bacc("TRN2", target_bir_lowering=False, debug=False)
    x = nc.dram_tensor("x", (N, FIN), F32, kind="ExternalInput").ap()
    w = nc.dram_tensor("w", (FIN, FOUT), F32, kind="ExternalInput").ap()
    a = nc.dram_tensor("a", (2 * FOUT, 1), F32, kind="ExternalInput").ap()
    # transposed output; the host un-transposes
    out = nc.dram_tensor("out", (FOUT, N), F32, kind="ExternalOutput").ap()
    with tile.TileContext(nc) as tc:
        _gat_body(tc, x, w, a, out)
    nc.compile()
    _NC_CACHE["nc"] = nc
    return nc


def kernel(x, W, a):
    x = np.ascontiguousarray(np.asarray(x), dtype=np.float32)
    W = np.ascontiguousarray(np.asarray(W), dtype=np.float32)
    a = np.ascontiguousarray(np.asarray(a), dtype=np.float32)
    assert x.shape == (N_CORES, N, FIN), x.shape
    nc = _build_nc()
    in_maps = [{"x": x[c], "w": W, "a": a} for c in range(N_CORES)]
    res = run_bass_kernel_spmd(nc, in_maps, core_ids=list(range(N_CORES)))
    return np.stack([res.results[c]["out"].T.copy() for c in range(N_CORES)], axis=0)


# revision 32
# speedup vs baseline: 1.0319x; 1.0319x over previous
"""GAT block (graph attention) Bass/Tile kernel for Trainium2, 8 NeuronCores.

Full-input contract: kernel(x=(8,2048,128), W=(128,64), a=(128,1)) -> (8,2048,64).
Sharding: data-parallel over batch — one batch element per core, W/a replicated.

Per-core math (N=2048, Fin=128, Fout=64):
  h  = x @ W                               (N, Fout)
  s1 = h @ a[:64, 0],  s2 = h @ a[64:, 0]  (N,)
  e[i, j]   = leakyrelu(s1[i] + s2[j], 0.2)
  att       = softmax(e, axis=0)  (normalize over i for each column j)
  out       = leakyrelu(att @ h, 0.2)

Implementation notes:
  * attention matrix kept transposed: Pt[j, i] = exp(lrelu(s1[i] + s2[j])).
    leakyrelu tiles are split across ACT and DVE(+GPSIMD) to balance engines;
    the exp runs on ACT (Prelu/parametric_relu shares the exp activation-table
    set so there are no table reloads) with accum_out giving the softmax
    denominator for free.
  * No max-subtraction: |s1+s2| <~ 15 so exp is far from fp32 overflow; this
    matches jax softmax to fp32 rounding.
  * setup matmuls run in float32r (single-pass) — fp32 matmuls on TRN2 are
    two-pass (LOW/HIGH) and twice the cost.  h and both score projections
    come from one stationary [W | W@a1 | W@a2] per x-tile.
  * out is accumulated transposed (hpT[f, i] in 4 PSUM banks, one per
    512-wide i-chunk) so the 64 bf16 matmuls overlap the ACT/DVE stream
    tile-by-tile; the host un-transposes the (64, 2048) result.
"""

import numpy as np
from contextlib import ExitStack

import concourse.bass as bass
import concourse.mybir as mybir
import concourse.tile as tile
from concourse.tile import add_dep_helper
from concourse import bacc
from concourse._compat import with_exitstack
from concourse.bass_utils import run_bass_kernel_spmd
from concourse.masks import make_identity

# ---- custom DVE op: out = max(in0 + s0, (in0 + s0) * imm2) — fused
# leakyrelu-with-per-partition-bias, one DVE instruction per tile ----
import numpy as _np
from concourse import dve_ops as _dvo
from concourse.dve_spec import Spec as _Spec, Src0 as _Src0, C0 as _C0, C2 as _C2
from concourse.dve_spec import maxx as _maxx, lower as _dve_lower
from concourse.dve_spec import _has_src1 as _dve_has_src1
from concourse.dve_uop import DveOpSpec as _DveOpSpec
from concourse.dve_table_gen import dve_ver_for as _dve_ver_for


def _register_lrelu_bias():
    name = "LRELU_BIAS_GAT_ANT"
    if name in _dvo._SUB_OPCODE_FOR_NAME:
        return next(o for o in _dvo.OPS if o.name == name)
    spec = _Spec(
        body=_maxx(_Src0 + _C0, (_Src0 + _C0) * _C2),
        reference=lambda in0, in1, s0, s1, imm2: _np.maximum(
            in0.astype(_np.float32) + s0, (in0.astype(_np.float32) + s0) * imm2
        ).astype(_np.float32),
    )
    op = _dvo.DveOp(name, spec, subdim=False, uops_sha={},
                    perf_en={"v3": True, "v4": True})
    row = _dvo._CUSTOM_DVE_ROW_BASE + len(_dvo.OPS)
    assert row < 0x20
    _dvo.OPS.append(op)
    _dvo.CUSTOM_DVE_SPECS[name] = spec
    _dvo._SUB_OPCODE_FOR_NAME[name] = row
    for ver in ("v3", "v4"):
        try:
            s = _DveOpSpec(name=name, opcode=row, uops=_dve_lower(spec, ver=ver),
                           rd1_en=_dve_has_src1(spec)).sha(ver)
            op.uops_sha[ver] = s
        except Exception:
            pass
    return op


_LRELU_BIAS = _register_lrelu_bias()

F32 = mybir.dt.float32
F32R = mybir.dt.float32r
BF16 = mybir.dt.bfloat16
AF = mybir.ActivationFunctionType
ALU = mybir.AluOpType

N = 2048
FIN = 128
FOUT = 64
P = 128
T = N // P          # 16 row tiles
NC = N // 512       # 4 i-chunks for the output accumulation
NEG_SLOPE = 0.2
N_CORES = 8


@with_exitstack
def _gat_body(ctx: ExitStack, tc: tile.TileContext, x, w, a, out):
    nc = tc.nc

    const = ctx.enter_context(tc.tile_pool(name="const", bufs=1))
    xin = ctx.enter_context(tc.tile_pool(name="xin", bufs=16))
    lpool = ctx.enter_context(tc.tile_pool(name="lrelu", bufs=6))
    dpool = ctx.enter_context(tc.tile_pool(name="denoms", bufs=2 * T))

    # ---- constants / persistent tiles ----
    ident = const.tile([P, P], F32)
    make_identity(nc, ident)
    w_raw = const.tile([FIN, FOUT], F32)
    nc.sync.dma_start(w_raw[:], w)
    a_raw = const.tile([FOUT, 2], F32)  # [:,0]=a1, [:,1]=a2
    nc.sync.dma_start(a_raw[:, 0:1], a[0:FOUT, :])
    nc.sync.dma_start(a_raw[:, 1:2], a[FOUT:, :])
    acol = const.tile([FOUT, 2], F32R)
    nc.vector.tensor_copy(acol[:], a_raw[:])
    ones_raw = const.tile([1, P], F32)
    nc.vector.memset(ones_raw[:], 1.0)
    ones_row = const.tile([1, P], F32R)
    nc.vector.tensor_copy(ones_row[:], ones_raw[:])

    xT = const.tile([P, T, P], F32R)        # x transposed: [k, t, n]
    hs12 = const.tile([P, T, FOUT + 2], F32)  # [h | s1 s2 cols] per tile
    hs_bf = const.tile([P, T, FOUT], BF16)  # h/denom in bf16
    wsa = const.tile([FIN, FOUT + 2], F32R)  # [W | W@a1 | W@a2]
    s1b = const.tile([P, N], F32)           # s1 broadcast along partitions
    srow = const.tile([2, N], F32R)         # [s1, s2] as rows
    p_all = const.tile([P, T, N], BF16)     # attention numerator, transposed
    o_sb = const.tile([FOUT, N], F32)       # output transposed

    with tc.tile_pool(name="ps_tr", bufs=2, space="PSUM") as ps_tr, \
         tc.tile_pool(name="ps_mm", bufs=2, space="PSUM") as ps_mm:
        # ~5us of junk bf16 matmuls so the PE HAM un-throttles (1.2->2.4GHz)
        # before the real setup matmuls; overlaps the x DMA wait.
        wup = const.tile([P, 512], BF16)
        nc.vector.memset(wup[:], 0.0)
        for i in range(7):
            ps_w = ps_mm.tile([P, 512], F32, tag="ps_h")
            nc.tensor.matmul(ps_w[:], lhsT=wup[:, 0:P], rhs=wup[:],
                             start=True, stop=True)

        # wsa = [W | W @ [a1, a2]]  (wa via wT = W.T, contraction over f)
        nc.vector.tensor_copy(wsa[:, 0:FOUT], w_raw[:])
        ps_wT = ps_mm.tile([FOUT, FIN], F32, tag="ps_h")
        nc.tensor.transpose(ps_wT[:], w_raw[:], ident[:])
        wT = const.tile([FOUT, FIN], F32R)
        nc.vector.tensor_copy(wT[:], ps_wT[:])
        ps_wa = ps_mm.tile([FIN, 2], F32, tag="ps_r")
        nc.tensor.matmul(ps_wa[:], lhsT=wT[:], rhs=acol[:], start=True, stop=True)
        nc.vector.tensor_copy(wsa[:, FOUT:], ps_wa[:])

        # Score path first (it gates the whole exp stream): per tile,
        # transpose; one tile behind, the tiny srow matmul + bcast chunks.
        # The h matmuls are emitted afterwards — they only matter mid-stream.
        score_done = {}

        def load_transpose(t):
            xn = xin.tile([P, FIN], F32, tag="xn", name=f"xn{t}")
            dma_eng = (nc.sync, nc.gpsimd)[t % 2]
            dma_eng.dma_start(xn[:], x[t * P:(t + 1) * P, :])
            psT = ps_tr.tile([P, P], F32, tag="ps_t", name=f"psT{t}")
            nc.tensor.transpose(psT[:], xn[:], ident[:])
            # f32->f32r rounding copy, split across ACT/DVE
            if t % 2 == 0:
                score_done["act"] = nc.scalar.copy(xT[:, t, :], psT[:])
            else:
                score_done["dve"] = nc.vector.tensor_copy(xT[:, t, :], psT[:])

        def srow_mm(t):
            ps_r = ps_mm.tile([2, P], F32, tag="ps_r", name=f"ps_r{t}")
            score_done["pe"] = nc.tensor.matmul(
                ps_r[:], lhsT=wsa[:, FOUT:], rhs=xT[:, t, :],
                start=True, stop=True)
            if t % 2 == 0:
                score_done["act"] = nc.scalar.copy(
                    srow[:, t * P:(t + 1) * P], ps_r[:])
            else:
                score_done["dve"] = nc.vector.tensor_copy(
                    srow[:, t * P:(t + 1) * P], ps_r[:])
            if t % 4 == 3:
                c = t // 4
                sl = slice(c * 512, (c + 1) * 512)
                ps_b = ps_mm.tile([P, 512], F32, tag="ps_b", name=f"ps_b{c}")
                score_done["pe"] = nc.tensor.matmul(
                    ps_b[:], lhsT=ones_row[:], rhs=srow[0:1, sl],
                    start=True, stop=True)
                score_done["dve"] = nc.vector.tensor_copy(s1b[:, sl], ps_b[:])

        load_transpose(0)
        for t in range(1, T):
            load_transpose(t)
            srow_mm(t - 1)
        srow_mm(T - 1)

        # h matmuls only matter mid-stream: keep them strictly after the
        # score path on each engine so they can't dilute its pipeline.
        for t in range(T):
            ps_h = ps_mm.tile([P, FOUT + 2], F32, tag="ps_h", name=f"ps_h{t}")
            mm = nc.tensor.matmul(ps_h[:], lhsT=xT[:, t, :], rhs=wsa[:],
                                  start=True, stop=True)
            add_dep_helper(mm.ins, score_done["pe"].ins, sync=False,
                           reason="h-matmuls after score path")
            if t % 2 == 0:
                cp = nc.scalar.copy(hs12[:, t, :], ps_h[:])
                add_dep_helper(cp.ins, score_done["act"].ins, sync=False,
                               reason="h-copies after xT casts")
            else:
                cp = nc.vector.tensor_copy(hs12[:, t, :], ps_h[:])
                add_dep_helper(cp.ins, score_done["dve"].ins, sync=False,
                               reason="h-copies after score path")

    # setup PSUM pools released; output accumulators take the banks
    ps_out = ctx.enter_context(tc.tile_pool(name="ps_out", bufs=1, space="PSUM"))
    hp_ps = [ps_out.tile([FOUT, 512], F32, tag=f"hp{c}", name=f"hp{c}")
             for c in range(NC)]

    # ---- main: per j-tile lrelu -> exp(+denom) -> scale h -> accumulate out ----
    GROUPS = [(0, 4), (4, 4), (8, 4), (12, 2), (14, 1), (15, 1)]
    for g0, gn in GROUPS:
        den_g = dpool.tile([P, gn], F32, tag="den", name=f"den{g0}")
        for t in range(g0, g0 + gn):
            s2c = hs12[:, t, FOUT + 1:FOUT + 2]
            l_t = lpool.tile([P, N], F32, tag="l")
            nc.vector._custom_dve(_LRELU_BIAS, out=l_t[:], in0=s1b[:],
                                  s0=s2c, imm2=NEG_SLOPE)
            nc.scalar.activation(p_all[:, t, :], l_t[:],
                                 AF.Exp, accum_out=den_g[:, t - g0:t - g0 + 1])

        rden_g = dpool.tile([P, gn], F32, tag="rden", name=f"rden{g0}")
        nc.vector.reciprocal(rden_g[:], den_g[:])
        for u in range(g0, g0 + gn):
            rd = rden_g[:, u - g0:u - g0 + 1]
            if u % 2 == 0:
                nc.scalar.activation(hs_bf[:, u, :], hs12[:, u, 0:FOUT],
                                     AF.Copy, scale=rd)
            else:
                nc.vector.tensor_scalar_mul(hs_bf[:, u, :],
                                            hs12[:, u, 0:FOUT], rd)
            for c in range(NC):
                nc.tensor.matmul(hp_ps[c][:], lhsT=hs_bf[:, u, :],
                                 rhs=p_all[:, u, c * 512:(c + 1) * 512],
                                 start=(u == 0), stop=(u == T - 1))

    # ---- epilogue: leakyrelu on ACT straight from PSUM, DMA out transposed ----
    for c in range(NC):
        sl = slice(c * 512, (c + 1) * 512)
        nc.scalar.activation(o_sb[:, sl], hp_ps[c][:], AF.Prelu,
                             bias=0.0, scale=1.0, alpha=NEG_SLOPE)
        nc.sync.dma_start(out[:, sl], o_sb[:, sl])


_NC_CACHE = {}


def _build_nc():
    if "nc" in _NC_CACHE:
        return _NC_CACHE["nc"]
    nc = bacc.Bacc("TRN2", target_bir_lowering=False, debug=False)
    x = nc.dram_tensor("x", (N, FIN), F32, kind="ExternalInput").ap()
    w = nc.dram_tensor("w", (FIN, FOUT), F32, kind="ExternalInput").ap()
    a = nc.dram_tensor("a", (2 * FOUT, 1), F32, kind="ExternalInput").ap()
    # transposed output; the host un-transposes
    out = nc.dram_tensor("out", (FOUT, N), F32, kind="ExternalOutput").ap()
    with tile.TileContext(nc) as tc:
        _gat_body(tc, x, w, a, out)
    nc.compile()
    _NC_CACHE["nc"] = nc
    return nc


def kernel(x, W, a):
    x = np.ascontiguousarray(np.asarray(x), dtype=np.float32)
    W = np.ascontiguousarray(np.asarray(W), dtype=np.float32)
    a = np.ascontiguousarray(np.asarray(a), dtype=np.float32)
    assert x.shape == (N_CORES, N, FIN), x.shape
    nc = _build_nc()
    in_maps = [{"x": x[c], "w": W, "a": a} for c in range(N_CORES)]
    res = run_bass_kernel_spmd(nc, in_maps, core_ids=list(range(N_CORES)))
    return np.stack([res.results[c]["out"].T.copy() for c in range(N_CORES)], axis=0)


# revision 33
# speedup vs baseline: 1.0368x; 1.0048x over previous
"""GAT block (graph attention) Bass/Tile kernel for Trainium2, 8 NeuronCores.

Full-input contract: kernel(x=(8,2048,128), W=(128,64), a=(128,1)) -> (8,2048,64).
Sharding: data-parallel over batch - one batch element per core, W/a replicated,
zero inter-core communication; host stacks (and un-transposes) per-core outputs.

Per-core math (N=2048, Fin=128, Fout=64):
  h  = x @ W                               (N, Fout)
  s1 = h @ a[:64, 0],  s2 = h @ a[64:, 0]  (N,)
  e[i, j] = leakyrelu(s1[i] + s2[j], 0.2)
  att     = softmax(e, axis=0)   (normalize over i for each column j)
  out     = leakyrelu(att @ h, 0.2)

Key implementation points:
  * The attention matrix is built transposed, Pt[j,i] = exp(lrelu(s1[i]+s2[j])),
    in 16 (128, 2048) row tiles.  Per tile exactly TWO big ops:
      - leakyrelu-with-bias on the Vector engine via a custom DVE op
        (out = max(in + s0, (in + s0) * 0.2), s0 = per-partition s2 column),
        registered at import time through concourse's custom-DVE table
        machinery;
      - Exp on the Scalar engine with accum_out, which yields the softmax
        denominator as a free fused reduction.
    The two streams pipeline tile-by-tile; exp (16 x 2us) is the critical
    engine.  No max-subtraction is needed: |s1+s2| <~ 15, far from fp32
    overflow, matching jax softmax to fp32 rounding.
  * s1 is broadcast across partitions with a K=1 PE matmul (ones (x) s1-row);
    s2 columns fall out of the same x-tile matmul that computes h by using a
    combined stationary [W | W@a1 | W@a2] (one f32r single-pass matmul per
    tile; plain fp32 PE matmuls are two-pass LOW/HIGH and twice the cost).
  * The output is accumulated transposed - hpT[f, i] in 4 PSUM banks, one per
    512-wide i-chunk - so the 64 bf16 matmuls (P and h/denom cast to bf16;
    errors average out over the j-contraction, end-to-end rel err ~1.3e-3)
    overlap the exp stream tile-by-tile.  Final leakyrelu runs on ACT
    directly from PSUM; the host transposes the (64, 2048) result back.
  * A short burst of junk bf16 matmuls at kernel start un-throttles the PE
    clock (HAM 1.2 -> 2.4 GHz) while the x DMAs land; x-tile transposes and
    the score path are emitted before the h matmuls so the exp stream starts
    as early as possible.
"""

import numpy as np
from contextlib import ExitStack

import concourse.bass as bass
import concourse.mybir as mybir
import concourse.tile as tile
from concourse.tile import add_dep_helper
from concourse import bacc
from concourse._compat import with_exitstack
from concourse.bass_utils import run_bass_kernel_spmd
from concourse.masks import make_identity

# ---- custom DVE op: out = max(in0 + s0, (in0 + s0) * imm2) — fused
# leakyrelu-with-per-partition-bias, one DVE instruction per tile ----
import numpy as _np
from concourse import dve_ops as _dvo
from concourse.dve_spec import Spec as _Spec, Src0 as _Src0, C0 as _C0, C2 as _C2
from concourse.dve_spec import maxx as _maxx, lower as _dve_lower
from concourse.dve_spec import _has_src1 as _dve_has_src1
from concourse.dve_uop import DveOpSpec as _DveOpSpec
from concourse.dve_table_gen import dve_ver_for as _dve_ver_for


def _register_lrelu_bias():
    name = "LRELU_BIAS_GAT_ANT"
    if name in _dvo._SUB_OPCODE_FOR_NAME:
        return next(o for o in _dvo.OPS if o.name == name)
    spec = _Spec(
        body=_maxx(_Src0 + _C0, (_Src0 + _C0) * _C2),
        reference=lambda in0, in1, s0, s1, imm2: _np.maximum(
            in0.astype(_np.float32) + s0, (in0.astype(_np.float32) + s0) * imm2
        ).astype(_np.float32),
    )
    op = _dvo.DveOp(name, spec, subdim=False, uops_sha={},
                    perf_en={"v3": True, "v4": True})
    row = _dvo._CUSTOM_DVE_ROW_BASE + len(_dvo.OPS)
    assert row < 0x20
    _dvo.OPS.append(op)
    _dvo.CUSTOM_DVE_SPECS[name] = spec
    _dvo._SUB_OPCODE_FOR_NAME[name] = row
    for ver in ("v3", "v4"):
        try:
            s = _DveOpSpec(name=name, opcode=row, uops=_dve_lower(spec, ver=ver),
                           rd1_en=_dve_has_src1(spec)).sha(ver)
            op.uops_sha[ver] = s
        except Exception:
            pass
    return op


_LRELU_BIAS = _register_lrelu_bias()

F32 = mybir.dt.float32
F32R = mybir.dt.float32r
BF16 = mybir.dt.bfloat16
AF = mybir.ActivationFunctionType
ALU = mybir.AluOpType

N = 2048
FIN = 128
FOUT = 64
P = 128
T = N // P          # 16 row tiles
NC = N // 512       # 4 i-chunks for the output accumulation
NEG_SLOPE = 0.2
N_CORES = 8


@with_exitstack
def _gat_body(ctx: ExitStack, tc: tile.TileContext, x, w, a, out):
    nc = tc.nc

    const = ctx.enter_context(tc.tile_pool(name="const", bufs=1))
    xin = ctx.enter_context(tc.tile_pool(name="xin", bufs=16))
    lpool = ctx.enter_context(tc.tile_pool(name="lrelu", bufs=6))
    dpool = ctx.enter_context(tc.tile_pool(name="denoms", bufs=2 * T))

    # ---- constants / persistent tiles ----
    ident = const.tile([P, P], F32)
    make_identity(nc, ident)
    w_raw = const.tile([FIN, FOUT], F32)
    nc.sync.dma_start(w_raw[:], w)
    a_raw = const.tile([FOUT, 2], F32)  # [:,0]=a1, [:,1]=a2
    nc.sync.dma_start(a_raw[:, 0:1], a[0:FOUT, :])
    nc.sync.dma_start(a_raw[:, 1:2], a[FOUT:, :])
    acol = const.tile([FOUT, 2], F32R)
    nc.vector.tensor_copy(acol[:], a_raw[:])
    ones_raw = const.tile([1, P], F32)
    nc.vector.memset(ones_raw[:], 1.0)
    ones_row = const.tile([1, P], F32R)
    nc.vector.tensor_copy(ones_row[:], ones_raw[:])

    xT = const.tile([P, T, P], F32R)        # x transposed: [k, t, n]
    hs12 = const.tile([P, T, FOUT + 2], F32)  # [h | s1 s2 cols] per tile
    hs_bf = const.tile([P, T, FOUT], BF16)  # h/denom in bf16
    wsa = const.tile([FIN, FOUT + 2], F32R)  # [W | W@a1 | W@a2]
    s1b = const.tile([P, N], F32)           # s1 broadcast along partitions
    srow = const.tile([2, N], F32R)         # [s1, s2] as rows
    p_all = const.tile([P, T, N], BF16)     # attention numerator, transposed
    o_sb = const.tile([FOUT, N], F32)       # output transposed

    with tc.tile_pool(name="ps_tr", bufs=2, space="PSUM") as ps_tr, \
         tc.tile_pool(name="ps_mm", bufs=2, space="PSUM") as ps_mm:
        # ~5us of junk bf16 matmuls so the PE HAM un-throttles (1.2->2.4GHz)
        # before the real setup matmuls; overlaps the x DMA wait.
        wup = const.tile([P, 512], BF16)
        nc.vector.memset(wup[:], 0.0)
        for i in range(7):
            ps_w = ps_mm.tile([P, 512], F32, tag="ps_h")
            nc.tensor.matmul(ps_w[:], lhsT=wup[:, 0:P], rhs=wup[:],
                             start=True, stop=True)

        # wsa = [W | W @ [a1, a2]]  (wa via wT = W.T, contraction over f)
        nc.vector.tensor_copy(wsa[:, 0:FOUT], w_raw[:])
        ps_wT = ps_mm.tile([FOUT, FIN], F32, tag="ps_h")
        nc.tensor.transpose(ps_wT[:], w_raw[:], ident[:])
        wT = const.tile([FOUT, FIN], F32R)
        nc.vector.tensor_copy(wT[:], ps_wT[:])
        ps_wa = ps_mm.tile([FIN, 2], F32, tag="ps_r")
        nc.tensor.matmul(ps_wa[:], lhsT=wT[:], rhs=acol[:], start=True, stop=True)
        nc.vector.tensor_copy(wsa[:, FOUT:], ps_wa[:])

        # Score path first (it gates the whole exp stream): per tile,
        # transpose; one tile behind, the tiny srow matmul + bcast chunks.
        # The h matmuls are emitted afterwards — they only matter mid-stream.
        score_done = {}

        def load_transpose(t):
            xn = xin.tile([P, FIN], F32, tag="xn", name=f"xn{t}")
            dma_eng = (nc.sync, nc.gpsimd)[t % 2]
            dma_eng.dma_start(xn[:], x[t * P:(t + 1) * P, :])
            psT = ps_tr.tile([P, P], F32, tag="ps_t", name=f"psT{t}")
            nc.tensor.transpose(psT[:], xn[:], ident[:])
            # f32->f32r rounding copy, split across ACT/DVE
            if t % 2 == 0:
                score_done["act"] = nc.scalar.copy(xT[:, t, :], psT[:])
            else:
                score_done["dve"] = nc.vector.tensor_copy(xT[:, t, :], psT[:])

        def srow_mm(t):
            ps_r = ps_mm.tile([2, P], F32, tag="ps_r", name=f"ps_r{t}")
            score_done["pe"] = nc.tensor.matmul(
                ps_r[:], lhsT=wsa[:, FOUT:], rhs=xT[:, t, :],
                start=True, stop=True)
            if t % 2 == 0:
                score_done["act"] = nc.scalar.copy(
                    srow[:, t * P:(t + 1) * P], ps_r[:])
            else:
                score_done["dve"] = nc.vector.tensor_copy(
                    srow[:, t * P:(t + 1) * P], ps_r[:])
            if t % 4 == 3:
                c = t // 4
                sl = slice(c * 512, (c + 1) * 512)
                ps_b = ps_mm.tile([P, 512], F32, tag="ps_b", name=f"ps_b{c}")
                score_done["pe"] = nc.tensor.matmul(
                    ps_b[:], lhsT=ones_row[:], rhs=srow[0:1, sl],
                    start=True, stop=True)
                score_done["dve"] = nc.vector.tensor_copy(s1b[:, sl], ps_b[:])

        load_transpose(0)
        for t in range(1, T):
            load_transpose(t)
            srow_mm(t - 1)
        srow_mm(T - 1)

        # h matmuls only matter mid-stream: keep them strictly after the
        # score path on each engine so they can't dilute its pipeline.
        for t in range(T):
            ps_h = ps_mm.tile([P, FOUT + 2], F32, tag="ps_h", name=f"ps_h{t}")
            mm = nc.tensor.matmul(ps_h[:], lhsT=xT[:, t, :], rhs=wsa[:],
                                  start=True, stop=True)
            add_dep_helper(mm.ins, score_done["pe"].ins, sync=False,
                           reason="h-matmuls after score path")
            if t % 2 == 0:
                cp = nc.scalar.copy(hs12[:, t, :], ps_h[:])
                add_dep_helper(cp.ins, score_done["act"].ins, sync=False,
                               reason="h-copies after xT casts")
            else:
                cp = nc.vector.tensor_copy(hs12[:, t, :], ps_h[:])
                add_dep_helper(cp.ins, score_done["dve"].ins, sync=False,
                               reason="h-copies after score path")

    # setup PSUM pools released; output accumulators take the banks
    ps_out = ctx.enter_context(tc.tile_pool(name="ps_out", bufs=1, space="PSUM"))
    hp_ps = [ps_out.tile([FOUT, 512], F32, tag=f"hp{c}", name=f"hp{c}")
             for c in range(NC)]

    # ---- main: per j-tile lrelu -> exp(+denom) -> scale h -> accumulate out ----
    GROUPS = [(0, 4), (4, 4), (8, 4), (12, 2), (14, 1), (15, 1)]
    for g0, gn in GROUPS:
        den_g = dpool.tile([P, gn], F32, tag="den", name=f"den{g0}")
        for t in range(g0, g0 + gn):
            s2c = hs12[:, t, FOUT + 1:FOUT + 2]
            l_t = lpool.tile([P, N], F32, tag="l")
            nc.vector._custom_dve(_LRELU_BIAS, out=l_t[:], in0=s1b[:],
                                  s0=s2c, imm2=NEG_SLOPE)
            nc.scalar.activation(p_all[:, t, :], l_t[:],
                                 AF.Exp, accum_out=den_g[:, t - g0:t - g0 + 1])

        rden_g = dpool.tile([P, gn], F32, tag="rden", name=f"rden{g0}")
        nc.vector.reciprocal(rden_g[:], den_g[:])
        for u in range(g0, g0 + gn):
            rd = rden_g[:, u - g0:u - g0 + 1]
            if u % 2 == 0:
                nc.scalar.activation(hs_bf[:, u, :], hs12[:, u, 0:FOUT],
                                     AF.Copy, scale=rd)
            else:
                nc.vector.tensor_scalar_mul(hs_bf[:, u, :],
                                            hs12[:, u, 0:FOUT], rd)
            for c in range(NC):
                nc.tensor.matmul(hp_ps[c][:], lhsT=hs_bf[:, u, :],
                                 rhs=p_all[:, u, c * 512:(c + 1) * 512],
                                 start=(u == 0), stop=(u == T - 1))

    # ---- epilogue: leakyrelu on ACT straight from PSUM, DMA out transposed ----
    for c in range(NC):
        sl = slice(c * 512, (c + 1) * 512)
        nc.scalar.activation(o_sb[:, sl], hp_ps[c][:], AF.Prelu,
                             bias=0.0, scale=1.0, alpha=NEG_SLOPE)
        nc.sync.dma_start(out[:, sl], o_sb[:, sl])


_NC_CACHE = {}


def _build_nc():
    if "nc" in _NC_CACHE:
        return _NC_CACHE["nc"]
    nc = bacc.Bacc("TRN2", target_bir_lowering=False, debug=False)
    x = nc.dram_tensor("x", (N, FIN), F32, kind="ExternalInput").ap()
    w = nc.dram_tensor("w", (FIN, FOUT), F32, kind="ExternalInput").ap()
    a = nc.dram_tensor("a", (2 * FOUT, 1), F32, kind="ExternalInput").ap()
    # transposed output; the host un-transposes
    out = nc.dram_tensor("out", (FOUT, N), F32, kind="ExternalOutput").ap()
    with tile.TileContext(nc) as tc:
        _gat_body(tc, x, w, a, out)
    nc.compile()
    _NC_CACHE["nc"] = nc
    return nc


def kernel(x, W, a):
    x = np.ascontiguousarray(np.asarray(x), dtype=np.float32)
    W = np.ascontiguousarray(np.asarray(W), dtype=np.float32)
    a = np.ascontiguousarray(np.asarray(a), dtype=np.float32)
    assert x.shape == (N_CORES, N, FIN), x.shape
    nc = _build_nc()
    in_maps = [{"x": x[c], "w": W, "a": a} for c in range(N_CORES)]
    res = run_bass_kernel_spmd(nc, in_maps, core_ids=list(range(N_CORES)))
    return np.stack([res.results[c]["out"].T.copy() for c in range(N_CORES)], axis=0)
